# revision 10
# baseline (speedup 1.0000x reference)
"""AnomalyTransformer layer on 8 TRN2 NeuronCores, data-parallel over batch.

Each core processes one batch element (B=8 == n_cores):
  - QKV projections + per-head series attention S (softmax) and prior P
    (row-normalized Gaussian), Z = S @ V, then LN -> MLP -> LN.
  - Outputs x_hat [N,D], P [H,N,N], S [H,N,N] per core; host stacks to full.

Layout strategy per core (N=D=512, H=8, dh=64, HID=2048, P=128 partitions):
  - Host passes x twice (natural [N,D] and transposed [D,N]) plus
    pre-transposed weights so every matmul contraction dim lands on
    partitions with no on-device weight transposes.
  - Matmuls run in bf16 (inputs cast on device, f32 PSUM accumulation);
    everything else (softmax scale, LN stats, normalizations, outputs)
    stays f32.
  - scores are computed twice (Q.K^T in [n,m] layout for softmax/S output,
    and K.Q^T in [m,n] layout so exp(scores^T) can feed the S@V matmul as
    the stationary operand) - cheaper than transposing E on-chip.
  - softmax skips max-subtraction (scores/8 are in [-2, 2] for this data);
    normalization folds into a per-partition scale after the S@V matmul.
  - P = gauss/rowsum: the 1/(sqrt(2pi) sigma) factor cancels in the row
    normalization, so P rows are exp(-d2 * 1/(2 sigma^2)) normalized.
  - Only exp/ln ACT funcs are used (softplus = ln(1+e^t), rsqrt =
    exp(-0.5 ln)), so one activation table set serves the whole kernel.
"""

import numpy as np

B, N, D, H, HID = 8, 512, 512, 8, 2048
DH = D // H          # 64
P = 128              # SBUF partitions
NCH = N // P         # 4 n-chunks
DCH = D // P         # 4 d/c-chunks
JCH = HID // P       # 16 hidden chunks

_BUILT = {}
LAST_RESULT = None   # BassKernelResults of the most recent run (for test.py)


def _split_multiwaits(nc):
    """This walrus build supports only one sync-wait command per
    instruction; move extra waits onto standalone NoOps placed before the
    instruction in the same engine stream (drains in the Tile tail carry
    up to 4)."""
    import concourse.mybir as mybir

    n_split = 0
    for f in nc.m.functions:
        for bb in f.blocks:
            new_insts = []
            for inst in bb.instructions:
                si = getattr(inst, "sync_info", None)
                if si and si.on_wait and len(si.on_wait) > 1:
                    waits = list(si.on_wait)
                    for w in waits[:-1]:
                        nop = mybir.InstNoOp(
                            name=f"{inst.name}_wsplit{n_split}",
                            ins=[], outs=[],
                            engine=inst.engine,
                            sync_info=mybir.SyncInfo(on_wait=[w], on_update=[]),
                        )
                        nop.bass_nofuse = True
                        new_insts.append(nop)
                        n_split += 1
                    inst.sync_info = mybir.SyncInfo(
                        on_wait=[waits[-1]], on_update=list(si.on_update or [])
                    )
                new_insts.append(inst)
            bb.instructions[:] = new_insts
    return n_split


def _build():
    import concourse.bass as bass
    import concourse.mybir as mybir
    import concourse.tile as tile
    from concourse.masks import make_identity
    from contextlib import ExitStack

    f32 = mybir.dt.float32
    bf16 = mybir.dt.bfloat16
    AF = mybir.ActivationFunctionType
    OP = mybir.AluOpType

    nc = bass.Bass(trn_type="TRN2", target_bir_lowering=False)

    # ---- DRAM parameters (per-core shard ABI; host prepares these) ----
    xT_d = nc.dram_tensor("xT", [D, N], f32, kind="ExternalInput")
    x_d = nc.dram_tensor("x", [N, D], f32, kind="ExternalInput")
    wqT_d = nc.dram_tensor("WqT", [D, D], f32, kind="ExternalInput")
    wkT_d = nc.dram_tensor("WkT", [D, D], f32, kind="ExternalInput")
    wvT_d = nc.dram_tensor("WvT", [D, D], f32, kind="ExternalInput")
    wsT_d = nc.dram_tensor("WsigT", [D, H], f32, kind="ExternalInput")
    w1T_d = nc.dram_tensor("W1T", [D, HID], f32, kind="ExternalInput")
    w2T_d = nc.dram_tensor("W2T", [HID, D], f32, kind="ExternalInput")
    bq_d = nc.dram_tensor("bq", [D], f32, kind="ExternalInput")
    bk_d = nc.dram_tensor("bk", [D], f32, kind="ExternalInput")
    bv_d = nc.dram_tensor("bv", [D], f32, kind="ExternalInput")
    bsig_d = nc.dram_tensor("bsig", [H], f32, kind="ExternalInput")
    b1_d = nc.dram_tensor("b1", [HID], f32, kind="ExternalInput")
    b2_d = nc.dram_tensor("b2", [D], f32, kind="ExternalInput")
    g1_d = nc.dram_tensor("ln1_g", [D], f32, kind="ExternalInput")
    be1_d = nc.dram_tensor("ln1_b", [D], f32, kind="ExternalInput")
    g2_d = nc.dram_tensor("ln2_g", [D], f32, kind="ExternalInput")
    be2_d = nc.dram_tensor("ln2_b", [D], f32, kind="ExternalInput")

    xhat_d = nc.dram_tensor("out_xhat", [N, D], f32, kind="ExternalOutput")
    P_d = nc.dram_tensor("out_P", [H, N, N], f32, kind="ExternalOutput")
    S_d = nc.dram_tensor("out_S", [H, N, N], f32, kind="ExternalOutput")

    idx = np.arange(N, dtype=np.float32)
    d2_np = (idx[:, None] - idx[None, :]) ** 2
    d2_d = nc.inline_tensor(d2_np, name="d2c")

    def bcast(dram_ap, parts=P):
        # [F] DRAM vector -> partition-broadcast AP for DMA into [parts, F]
        return bass.AP(
            tensor=dram_ap.tensor,
            offset=dram_ap.offset,
            ap=[[0, parts]] + list(dram_ap.ap),
        )

    with tile.TileContext(nc) as tc, ExitStack() as ctx:
        pool1 = ctx.enter_context(tc.tile_pool(name="singles", bufs=1))
        wstage = ctx.enter_context(tc.tile_pool(name="wstage", bufs=2))

        # --- persistent small tiles ---
        ident = pool1.tile([P, P], bf16)
        make_identity(nc, ident)
        eps_t = pool1.tile([P, 1], f32)
        nc.vector.memset(eps_t, 1e-5)
        ones_row = pool1.tile([1, N], bf16)
        nc.vector.memset(ones_row, 1.0)
        # f32 staging rows -> bf16 rows for the bias outer products
        brow_f = pool1.tile([1, 3 * D], f32)
        nc.sync.dma_start(out=brow_f[:, 0:D], in_=bv_d[None, :])
        nc.sync.dma_start(out=brow_f[:, D : D + H], in_=bsig_d[None, :])
        nc.sync.dma_start(out=brow_f[:, 2 * D : 3 * D], in_=b2_d[None, :])
        brow_b = pool1.tile([1, 3 * D], bf16)
        nc.vector.tensor_copy(out=brow_b, in_=brow_f)
        bvrow = brow_b[:, 0:D]
        bsrow = brow_b[:, D : D + H]
        b2row = brow_b[:, 2 * D : 3 * D]

        bq_t = pool1.tile([P, DCH], f32)
        bk_t = pool1.tile([P, DCH], f32)
        b1_t = pool1.tile([P, JCH], f32)
        for c in range(DCH):
            nc.sync.dma_start(out=bq_t[:, c : c + 1], in_=bq_d[c * P : (c + 1) * P, None])
            nc.sync.dma_start(out=bk_t[:, c : c + 1], in_=bk_d[c * P : (c + 1) * P, None])
        for c in range(JCH):
            nc.sync.dma_start(out=b1_t[:, c : c + 1], in_=b1_d[c * P : (c + 1) * P, None])
        g1_t = pool1.tile([P, D], f32)
        be1_t = pool1.tile([P, D], f32)
        g2_t = pool1.tile([P, D], f32)
        be2_t = pool1.tile([P, D], f32)
        nc.sync.dma_start(out=g1_t, in_=bcast(g1_d[:]))
        nc.sync.dma_start(out=be1_t, in_=bcast(be1_d[:]))
        nc.sync.dma_start(out=g2_t, in_=bcast(g2_d[:]))
        nc.sync.dma_start(out=be2_t, in_=bcast(be2_d[:]))

        d2_t = pool1.tile([P, NCH, N], f32)
        for c in range(NCH):
            nc.sync.dma_start(out=d2_t[:, c, :], in_=d2_d[c * P : (c + 1) * P, :])

        # --- resident activations (bf16 ones feed the PE) ---
        xtb_t = pool1.tile([P, DCH, N], bf16)  # x^T
        qt_t = pool1.tile([P, DCH, N], bf16)   # Q^T: [d, n]
        kt_t = pool1.tile([P, DCH, N], bf16)   # K^T: [d, n]
        v_t = pool1.tile([P, NCH, D], bf16)    # V:   [m, d]
        w1b_t = pool1.tile([P, DCH, HID], bf16)
        w2b_t = pool1.tile([P, JCH, D], bf16)
        nsig_t = pool1.tile([P, NCH, H], f32)  # -1/(2 sigma^2), [n, h]
        rinv_t = pool1.tile([P, NCH, H], f32)  # 1/rowsum(E), [n, h]
        zx_t = pool1.tile([P, NCH, D], f32)    # Zh + x
        z_t = pool1.tile([P, NCH, D], f32)     # LN1 output
        zb_t = pool1.tile([P, NCH, D], bf16)   # LN1 output, bf16
        zt_t = pool1.tile([P, DCH, N], bf16)   # Z^T

        # MLP weights: DMA f32 chunks into staging, cast to resident bf16.
        # Issued up-front so the DMA + cast overlap the attention phase.
        for c in range(DCH):
            w1s = wstage.tile([P, HID], f32, tag="wst")
            nc.sync.dma_start(out=w1s, in_=w1T_d[c * P : (c + 1) * P, :])
            nc.vector.tensor_copy(out=w1b_t[:, c, :], in_=w1s)
        for c in range(JCH):
            w2s = wstage.tile([P, HID], f32, tag="wst")
            nc.sync.dma_start(out=w2s[:, :D], in_=w2T_d[c * P : (c + 1) * P, :])
            nc.vector.tensor_copy(out=w2b_t[:, c, :], in_=w2s[:, :D])

        # ---------------- Phase 1: QKV + sigma ----------------
        with ExitStack() as pctx:
            wpool = pctx.enter_context(tc.tile_pool(name="wqkv", bufs=1))
            pp = pctx.enter_context(tc.tile_pool(name="ps_qkv", bufs=2, space="PSUM"))
            pps = pctx.enter_context(tc.tile_pool(name="ps_sig", bufs=2, space="PSUM"))

            xt_t = wpool.tile([P, DCH, N], f32, tag="xt")
            for c in range(DCH):
                nc.sync.dma_start(out=xt_t[:, c, :], in_=xT_d[c * P : (c + 1) * P, :])
            wq_t = wpool.tile([P, DCH, D], f32, tag="wq")
            wk_t = wpool.tile([P, DCH, D], f32, tag="wk")
            wv_t = wpool.tile([P, DCH, D], f32, tag="wv")
            ws_t = wpool.tile([P, DCH, H], f32, tag="ws")
            wqb = wpool.tile([P, DCH, D], bf16, tag="wqb")
            wkb = wpool.tile([P, DCH, D], bf16, tag="wkb")
            wvb = wpool.tile([P, DCH, D], bf16, tag="wvb")
            wsb = wpool.tile([P, DCH, H], bf16, tag="wsb")
            for c in range(DCH):
                nc.sync.dma_start(out=wq_t[:, c, :], in_=wqT_d[c * P : (c + 1) * P, :])
                nc.sync.dma_start(out=wk_t[:, c, :], in_=wkT_d[c * P : (c + 1) * P, :])
                nc.sync.dma_start(out=wv_t[:, c, :], in_=wvT_d[c * P : (c + 1) * P, :])
                nc.sync.dma_start(out=ws_t[:, c, :], in_=wsT_d[c * P : (c + 1) * P, :])
                nc.vector.tensor_copy(out=xtb_t[:, c, :], in_=xt_t[:, c, :])
                nc.vector.tensor_copy(out=wqb[:, c, :], in_=wq_t[:, c, :])
                nc.vector.tensor_copy(out=wkb[:, c, :], in_=wk_t[:, c, :])
                nc.vector.tensor_copy(out=wvb[:, c, :], in_=wv_t[:, c, :])
                nc.vector.tensor_copy(out=wsb[:, c, :], in_=ws_t[:, c, :])

            # Q^T[d,n] = sum_c WqT[c,d] * xT[c,n]; bias added at evacuation
            for dc in range(DCH):
                for w, bias, dst in ((wqb, bq_t, qt_t), (wkb, bk_t, kt_t)):
                    ps = pp.tile([P, N], f32, tag="ps_proj")
                    for cc in range(DCH):
                        nc.tensor.matmul(
                            ps,
                            lhsT=w[:, cc, dc * P : (dc + 1) * P],
                            rhs=xtb_t[:, cc, :],
                            start=(cc == 0),
                            stop=(cc == DCH - 1),
                        )
                    nc.scalar.activation(
                        out=dst[:, dc, :], in_=ps, func=AF.Identity,
                        bias=bias[:, dc : dc + 1],
                    )
                # V[m,d] = sum_c xT[c,m] * WvT[c,d] + bv[d]
                ps = pp.tile([P, D], f32, tag="ps_proj")
                for cc in range(DCH):
                    nc.tensor.matmul(
                        ps,
                        lhsT=xtb_t[:, cc, dc * P : (dc + 1) * P],
                        rhs=wvb[:, cc, :],
                        start=(cc == 0),
                        stop=False,
                    )
                nc.tensor.matmul(
                    ps, lhsT=ones_row[:, :P], rhs=bvrow,
                    start=False, stop=True,
                )
                nc.vector.tensor_copy(out=v_t[:, dc, :], in_=ps)

                # sigma[n,h]; then -1/(2 sigma^2).
                # softplus(t) = ln(1 + e^t): only exp/ln ACT funcs kernel-wide
                ps2 = pps.tile([P, H], f32, tag="ps_sig")
                for cc in range(DCH):
                    nc.tensor.matmul(
                        ps2,
                        lhsT=xtb_t[:, cc, dc * P : (dc + 1) * P],
                        rhs=wsb[:, cc, :],
                        start=(cc == 0),
                        stop=False,
                    )
                nc.tensor.matmul(
                    ps2, lhsT=ones_row[:, :P], rhs=bsrow,
                    start=False, stop=True,
                )
                sg = nsig_t[:, dc, :]
                nc.scalar.activation(out=sg, in_=ps2, func=AF.Exp)
                nc.vector.tensor_scalar_add(out=sg, in0=sg, scalar1=1.0)
                nc.scalar.activation(out=sg, in_=sg, func=AF.Ln)
                nc.vector.tensor_scalar_add(out=sg, in0=sg, scalar1=1e-5)
                nc.vector.tensor_mul(out=sg, in0=sg, in1=sg)
                nc.vector.reciprocal(out=sg, in_=sg)
                nc.vector.tensor_scalar_mul(out=sg, in0=sg, scalar1=-0.5)

        # ---------------- Phase 2: attention + P, per head ----------------
        with ExitStack() as pctx:
            ps_sc = pctx.enter_context(tc.tile_pool(name="ps_sc", bufs=2, space="PSUM"))
            ps_sct = pctx.enter_context(tc.tile_pool(name="ps_sct", bufs=2, space="PSUM"))
            ps_zh = pctx.enter_context(tc.tile_pool(name="ps_zh", bufs=1, space="PSUM"))
            spool = pctx.enter_context(tc.tile_pool(name="sp", bufs=3))
            etpool = pctx.enter_context(tc.tile_pool(name="etp", bufs=2))
            small = pctx.enter_context(tc.tile_pool(name="sm", bufs=4))

            zh_ps = [
                ps_zh.tile([P, D], f32, tag=f"zh{c}", name=f"zh{c}")
                for c in range(NCH)
            ]

            for h in range(H):
                hb = 64 * (h % 2)
                hc = h // 2
                k_h = kt_t[hb : hb + 64, hc, :]
                q_h = qt_t[hb : hb + 64, hc, :]

                # S side: scores [n, m] -> E -> rowsum -> S out
                for ncc in range(NCH):
                    ps = ps_sc.tile([P, N], f32, tag="sc")
                    nc.tensor.matmul(
                        ps,
                        lhsT=qt_t[hb : hb + 64, hc, ncc * P : (ncc + 1) * P],
                        rhs=k_h,
                        start=True, stop=True,
                    )
                    e_t = spool.tile([P, N], f32, tag="e")
                    rsum = small.tile([P, 1], f32, tag="rsum")
                    nc.scalar.activation(
                        out=e_t, in_=ps, func=AF.Exp, scale=0.125,
                        accum_out=rsum,
                    )
                    rv = rinv_t[:, ncc, h : h + 1]
                    nc.vector.reciprocal(out=rv, in_=rsum)
                    s_t = spool.tile([P, N], f32, tag="s")
                    nc.vector.tensor_scalar_mul(out=s_t, in0=e_t, scalar1=rv)
                    nc.sync.dma_start(
                        out=S_d[h, ncc * P : (ncc + 1) * P, :], in_=s_t
                    )

                # T side: scores^T [m, n] -> E^T (stationary for S@V)
                et_t = etpool.tile([P, NCH, N], bf16, tag="et")
                for mc in range(NCH):
                    ps = ps_sct.tile([P, N], f32, tag="sct")
                    nc.tensor.matmul(
                        ps,
                        lhsT=kt_t[hb : hb + 64, hc, mc * P : (mc + 1) * P],
                        rhs=q_h,
                        start=True, stop=True,
                    )
                    nc.scalar.activation(
                        out=et_t[:, mc, :], in_=ps, func=AF.Exp, scale=0.125
                    )

                # Zh columns for this head: sum_m E^T[m,n] V[m, hd]
                for ncc in range(NCH):
                    for mc in range(NCH):
                        nc.tensor.matmul(
                            zh_ps[ncc][:, h * DH : (h + 1) * DH],
                            lhsT=et_t[:, mc, ncc * P : (ncc + 1) * P],
                            rhs=v_t[:, mc, h * DH : (h + 1) * DH],
                            start=(mc == 0),
                            stop=(mc == NCH - 1),
                        )

                # P side: gauss = exp(-d2/(2 sig^2)), row-normalized
                for ncc in range(NCH):
                    g_t = spool.tile([P, N], f32, tag="g")
                    gsum = small.tile([P, 1], f32, tag="gsum")
                    nc.scalar.activation(
                        out=g_t, in_=d2_t[:, ncc, :], func=AF.Exp,
                        scale=nsig_t[:, ncc, h : h + 1],
                        accum_out=gsum,
                    )
                    ginv = small.tile([P, 1], f32, tag="ginv")
                    nc.vector.reciprocal(out=ginv, in_=gsum)
                    p_t = spool.tile([P, N], f32, tag="p")
                    nc.vector.tensor_scalar_mul(out=p_t, in0=g_t, scalar1=ginv)
                    nc.sync.dma_start(
                        out=P_d[h, ncc * P : (ncc + 1) * P, :], in_=p_t
                    )

            # Zh normalize + residual: zx = Zh * rinv + x
            with tc.tile_pool(name="xres", bufs=2) as xrp:
                for ncc in range(NCH):
                    x_t = xrp.tile([P, D], f32, tag="x")
                    nc.sync.dma_start(
                        out=x_t, in_=x_d[ncc * P : (ncc + 1) * P, :]
                    )
                    for h in range(H):
                        nc.vector.scalar_tensor_tensor(
                            out=zx_t[:, ncc, h * DH : (h + 1) * DH],
                            in0=zh_ps[ncc][:, h * DH : (h + 1) * DH],
                            scalar=rinv_t[:, ncc, h : h + 1],
                            in1=x_t[:, h * DH : (h + 1) * DH],
                            op0=OP.mult, op1=OP.add,
                        )

        # ---------------- Phase 3: LN1 + transpose ----------------
        with ExitStack() as pctx:
            st = pctx.enter_context(tc.tile_pool(name="stats", bufs=4))
            ps_tr = pctx.enter_context(tc.tile_pool(name="ps_tr", bufs=2, space="PSUM"))
            for ncc in range(NCH):
                stat = st.tile([P, 6], f32, tag="bn")
                mv = st.tile([P, 2], f32, tag="mv")
                nc.vector.bn_stats(out=stat, in_=zx_t[:, ncc, :])
                nc.vector.bn_aggr(out=mv, in_=stat)
                # rstd = exp(-0.5 ln(var + eps))
                rstd = st.tile([P, 1], f32, tag="rstd")
                nc.scalar.activation(
                    out=rstd, in_=mv[:, 1:2], func=AF.Ln, bias=eps_t
                )
                nc.scalar.activation(out=rstd, in_=rstd, func=AF.Exp, scale=-0.5)
                zn = z_t[:, ncc, :]
                nc.vector.tensor_scalar(
                    out=zn, in0=zx_t[:, ncc, :],
                    scalar1=mv[:, 0:1], scalar2=rstd,
                    op0=OP.subtract, op1=OP.mult,
                )
                nc.vector.tensor_mul(out=zn, in0=zn, in1=g1_t)
                nc.vector.tensor_add(out=zn, in0=zn, in1=be1_t)
                nc.vector.tensor_copy(out=zb_t[:, ncc, :], in_=zn)
            # Z^T via PE transpose (bf16)
            for dc in range(DCH):
                for ncc in range(NCH):
                    pst = ps_tr.tile([P, P], bf16, tag="tr")
                    nc.tensor.transpose(
                        pst, in_=zb_t[:, ncc, dc * P : (dc + 1) * P], identity=ident
                    )
                    nc.vector.tensor_copy(
                        out=zt_t[:, dc, ncc * P : (ncc + 1) * P], in_=pst
                    )

        # ---------------- Phase 4: MLP + LN2 ----------------
        with ExitStack() as pctx:
            hp = pctx.enter_context(tc.tile_pool(name="hid", bufs=1))
            ps_h = pctx.enter_context(tc.tile_pool(name="ps_h", bufs=3, space="PSUM"))
            ps_x = pctx.enter_context(tc.tile_pool(name="ps_x", bufs=2, space="PSUM"))
            op_ = pctx.enter_context(tc.tile_pool(name="outp", bufs=3))
            st = pctx.enter_context(tc.tile_pool(name="stats2", bufs=4))

            hid_t = hp.tile([P, JCH, N], bf16, tag="hid")
            for jc in range(JCH):
                ps = ps_h.tile([P, N], f32, tag="ph")
                for dc in range(DCH):
                    nc.tensor.matmul(
                        ps,
                        lhsT=w1b_t[:, dc, jc * P : (jc + 1) * P],
                        rhs=zt_t[:, dc, :],
                        start=(dc == 0),
                        stop=(dc == DCH - 1),
                    )
                # hidden^T = relu(. + b1)
                nc.vector.tensor_scalar(
                    out=hid_t[:, jc, :], in0=ps,
                    scalar1=b1_t[:, jc : jc + 1], scalar2=0.0,
                    op0=OP.add, op1=OP.max,
                )

            for ncc in range(NCH):
                ps = ps_x.tile([P, D], f32, tag="px")
                for jc in range(JCH):
                    nc.tensor.matmul(
                        ps,
                        lhsT=hid_t[:, jc, ncc * P : (ncc + 1) * P],
                        rhs=w2b_t[:, jc, :],
                        start=(jc == 0),
                        stop=False,
                    )
                nc.tensor.matmul(
                    ps, lhsT=ones_row[:, :P], rhs=b2row,
                    start=False, stop=True,
                )
                # residual
                xh = op_.tile([P, D], f32, tag="xh")
                nc.vector.scalar_tensor_tensor(
                    out=xh, in0=ps, scalar=1.0, in1=z_t[:, ncc, :],
                    op0=OP.mult, op1=OP.add,
                )
                stat = st.tile([P, 6], f32, tag="bn2")
                mv = st.tile([P, 2], f32, tag="mv2")
                nc.vector.bn_stats(out=stat, in_=xh)
                nc.vector.bn_aggr(out=mv, in_=stat)
                rstd = st.tile([P, 1], f32, tag="rstd2")
                nc.scalar.activation(
                    out=rstd, in_=mv[:, 1:2], func=AF.Ln, bias=eps_t
                )
                nc.scalar.activation(out=rstd, in_=rstd, func=AF.Exp, scale=-0.5)
                o_t = op_.tile([P, D], f32, tag="o")
                nc.vector.tensor_scalar(
                    out=o_t, in0=xh,
                    scalar1=mv[:, 0:1], scalar2=rstd,
                    op0=OP.subtract, op1=OP.mult,
                )
                nc.vector.tensor_mul(out=o_t, in0=o_t, in1=g2_t)
                nc.vector.tensor_add(out=o_t, in0=o_t, in1=be2_t)
                nc.sync.dma_start(
                    out=xhat_d[ncc * P : (ncc + 1) * P, :], in_=o_t
                )

    _split_multiwaits(nc)
    return nc


def kernel(x, Wq, bq, Wk, bk, Wv, bv, Wsig, bsig,
           ln1_g, ln1_b, W1, b1, W2, b2, ln2_g, ln2_b):
    global LAST_RESULT
    import os
    from concourse.bass_utils import run_bass_kernel_spmd

    if "nc" not in _BUILT:
        _BUILT["nc"] = _build()
    nc = _BUILT["nc"]

    f = np.ascontiguousarray
    x = np.asarray(x, dtype=np.float32)
    shared = dict(
        WqT=f(np.asarray(Wq, np.float32).T), bq=f(np.asarray(bq, np.float32)),
        WkT=f(np.asarray(Wk, np.float32).T), bk=f(np.asarray(bk, np.float32)),
        WvT=f(np.asarray(Wv, np.float32).T), bv=f(np.asarray(bv, np.float32)),
        WsigT=f(np.asarray(Wsig, np.float32).T), bsig=f(np.asarray(bsig, np.float32)),
        W1T=f(np.asarray(W1, np.float32).T), b1=f(np.asarray(b1, np.float32)),
        W2T=f(np.asarray(W2, np.float32).T), b2=f(np.asarray(b2, np.float32)),
        ln1_g=f(np.asarray(ln1_g, np.float32)), ln1_b=f(np.asarray(ln1_b, np.float32)),
        ln2_g=f(np.asarray(ln2_g, np.float32)), ln2_b=f(np.asarray(ln2_b, np.float32)),
    )
    in_maps = []
    for b in range(B):
        xb = f(x[b])
        in_maps.append(dict(shared, x=xb, xT=f(xb.T)))

    trace = bool(int(os.environ.get("KERNEL_TRACE", "0")))
    res = run_bass_kernel_spmd(
        nc, in_maps, core_ids=list(range(B)), trace=trace
    )
    LAST_RESULT = res
    rs = res.results
    x_hat = np.stack([r["out_xhat"] for r in rs])
    P_out = np.stack([r["out_P"] for r in rs])
    S_out = np.stack([r["out_S"] for r in rs])
    return x_hat, P_out, S_out
